# revision 1
# baseline (speedup 1.0000x reference)
"""Trainium2 Bass kernel for nn_DiffLogicPBF (difflogic network).

Algorithm
---------
The network input is binarized to 2 bits, so every batch row's entire
activation trajectory takes one of only 4 values ("patterns").  We evaluate
the network on the 4 patterns instead of 8192 rows, then blend per-row.

The per-layer gathers (connection indices) are known when the kernel is
built, so they are composed on the host into a stream tree: layer l needs
its layer-(l-1) inputs in 2 permuted orders, giving 2^(5-l) "streams" per
layer (63 total), each a gather-free elementwise evaluation.  Weights are
uploaded pre-permuted per stream; softmax/logic-coefficient math runs on
device via exp + strided corner-mask reductions (the 16 soft logic
functions' truth tables at the 4 binary corners are exact bit masks).

Sharding: neurons (K=4096) are split across the 8 cores (512 each).  Each
core computes its partial GroupSum table [4 patterns x 2 classes], blends
the full batch against it ([B,2] partial logits), and the host sums the 8
partial outputs (the blend is linear in the table).

Engine split: DVE does the corner reductions and the multilinear eval;
GpSimd does the coefficient algebra and the a*b products of the two big
layers; ACT does exp; PE broadcasts the table.  The weight blobs arrive in
4 contiguous chunks so exp/reduces pipeline with the DMA.
"""

from contextlib import ExitStack

import ml_dtypes
import numpy as np

import concourse.bacc as bacc
import concourse.bass as bass
import concourse.mybir as mybir
import concourse.tile as tile
from concourse.bass_utils import run_bass_kernel_spmd

F32 = mybir.dt.float32
ADD = mybir.AluOpType.add
SUB = mybir.AluOpType.subtract
MUL = mybir.AluOpType.mult
X = mybir.AxisListType.X
XY = mybir.AxisListType.XY

N_CORES = 8
B, K, L = 8192, 4096, 6
NS = [32, 16, 8, 4, 2, 1]          # streams per layer
NSTOT = sum(NS)                    # 63
KLOC = K // N_CORES                # 512 neurons per core
J = KLOC // 128                    # 4 free chunks per partition
FO = np.cumsum([0] + NS).tolist()  # stream offsets by layer
NSJ = NSTOT * J                    # 252
BROW = B // 128                    # 64 batch rows per partition

# weight pipeline groups as (start_stream, n_streams); layer 0 is split in
# half so the first exp/reduce chunk starts after ~0.5 MB of DMA
WG = [(0, 32), (32, 16), (48, 15)]

_compiled = None


def _build_program():
    nc = bacc.Bacc("TRN2", target_bir_lowering=False, debug=False,
                   num_devices=N_CORES)
    BF16 = mybir.dt.bfloat16
    walls = [nc.dram_tensor(f"wall{gi}", [128, n * J * 16], BF16,
                            kind="ExternalInput").ap()
             for gi, (s0_, n) in enumerate(WG)]
    a0in = nc.dram_tensor("a0in", [128, NS[0] * J * 4], mybir.dt.bfloat16, kind="ExternalInput").ap()
    b0in = nc.dram_tensor("b0in", [128, NS[0] * J * 4], mybir.dt.bfloat16, kind="ExternalInput").ap()
    xin = nc.dram_tensor("xin", [128, BROW, 2], F32, kind="ExternalInput").ap()
    clsg = nc.dram_tensor("clsg", [128, 2], F32, kind="ExternalInput").ap()
    out = nc.dram_tensor("out", [B, 2], F32, kind="ExternalOutput").ap()

    EXP = mybir.ActivationFunctionType.Exp
    GT = mybir.AluOpType.is_gt

    with tile.TileContext(nc) as tc:
        with ExitStack() as ctx:
            p = ctx.enter_context(tc.tile_pool(name="p", bufs=1))
            psp = ctx.enter_context(tc.tile_pool(name="ps", bufs=1, space="PSUM"))

            # ---- input DMAs (split across trigger engines / queues) ----
            dma_engines = [nc.sync, nc.scalar, nc.sync, nc.scalar]
            xt = p.tile([128, BROW, 2], F32)
            nc.scalar.dma_start(xt[:], xin[:])
            ct = p.tile([128, 2], F32)
            nc.scalar.dma_start(ct[:], clsg[:])
            wts = []
            for gi, (s0_, n) in enumerate(WG):
                wt = p.tile([128, n * J * 16], BF16, tag=f"wt{gi}")
                nc.sync.dma_start(wt[:], walls[gi][:])
                wts.append(wt)
                if gi == 0:
                    a0t = p.tile([128, NS[0] * J * 4], BF16)
                    nc.scalar.dma_start(a0t[:], a0in[:])
                    b0t = p.tile([128, NS[0] * J * 4], BF16)
                    nc.scalar.dma_start(b0t[:], b0in[:])

            # blend prep + constants (fill early DVE idle time)
            s0 = p.tile([128, BROW], F32)
            nc.vector.tensor_scalar(s0[:], xt[:, :, 0], 0.0, None, op0=GT)
            s1 = p.tile([128, BROW], F32)
            nc.vector.tensor_scalar(s1[:], xt[:, :, 1], 0.0, None, op0=GT)
            t01 = p.tile([128, BROW], F32)
            nc.vector.tensor_tensor(t01[:], s0[:], s1[:], op=MUL)
            ones_m = p.tile([128, 128], F32)
            nc.vector.memset(ones_m[:], 1.0)
            # warm the PE early: 1x1 matmul of ones written back into ones_m
            # (semantically a no-op, keeps the chain live through the real
            # matmul below)
            wm = psp.tile([1, 1], F32)
            nc.tensor.matmul(wm[:], ones_m[0:1, 0:1], ones_m[0:1, 0:1],
                             start=True, stop=True)
            nc.scalar.copy(ones_m[0:1, 0:1], wm[:])

            # ---- per-group: exp -> corner masks (DVE) -> coeffs (GpSimd) ----
            Cw = []                      # per-wgroup (C0..C3) tiles
            Dw, rw, Tw = [], [], []
            for gi, (s0_, n) in enumerate(WG):
                nsjg = n * J
                E = p.tile([128, nsjg * 16], F32, tag=f"E{gi}")
                nc.scalar.activation(E[:], wts[gi][:], EXP)
                Ev = E[:].rearrange("p (n i) -> p n i", i=16)
                e0 = Ev[:, :, 0:1]

                rd = nc.vector.tensor_reduce
                gt = nc.gpsimd.tensor_tensor
                V11 = p.tile([128, nsjg], F32, tag=f"V11{gi}")
                rd(V11[:], Ev[:, :, 1::2], axis=X, op=ADD)
                Sev = p.tile([128, nsjg], F32, tag=f"Sev{gi}")
                rd(Sev[:], Ev[:, :, 0::2], axis=X, op=ADD)
                D = p.tile([128, nsjg], F32, tag=f"D{gi}")
                nc.vector.tensor_tensor(D[:], V11[:], Sev[:], op=ADD)
                Dw.append(D)
                r = p.tile([128, nsjg], F32, tag=f"r{gi}")
                rw.append(r)
                V10 = p.tile([128, nsjg], F32, tag=f"V10{gi}")
                m10 = bass.AP(tensor=e0.tensor, offset=e0.offset + 2,
                              ap=[e0.ap[0], [16, nsjg], [4, 4], [1, 2]])
                rd(V10[:], m10, axis=XY, op=ADD)
                V01 = p.tile([128, nsjg], F32, tag=f"V01{gi}")
                m01 = bass.AP(tensor=e0.tensor, offset=e0.offset + 4,
                              ap=[e0.ap[0], [16, nsjg], [8, 2], [1, 4]])
                rd(V01[:], m01, axis=XY, op=ADD)
                V00 = p.tile([128, nsjg], F32, tag=f"V00{gi}")
                rd(V00[:], Ev[:, :, 8:16], axis=X, op=ADD)

                t1 = p.tile([128, nsjg], F32, tag=f"t1{gi}")
                gt(t1[:], V11[:], V10[:], op=SUB)
                t2 = p.tile([128, nsjg], F32, tag=f"t2{gi}")
                gt(t2[:], V01[:], V00[:], op=SUB)
                t3 = p.tile([128, nsjg], F32, tag=f"t3{gi}")
                gt(t3[:], V10[:], V00[:], op=SUB)
                c3u = p.tile([128, nsjg], F32, tag=f"c3u{gi}")
                gt(c3u[:], t1[:], t2[:], op=SUB)
                Tw.append((V00, t3, t2, c3u))

            # 1/D on ACT via exp(-ln(D))
            lns = []
            for gi, (s0_, n) in enumerate(WG):
                nsjg = n * J
                lnD = p.tile([128, nsjg], F32, tag=f"lnD{gi}")
                nc.scalar.activation(lnD[:], Dw[gi][:],
                                     mybir.ActivationFunctionType.Ln)
                lns.append(lnD)
            for gi, (s0_, n) in enumerate(WG):
                nc.scalar.activation(rw[gi][:], lns[gi][:], EXP, scale=-1.0)

            for gi, (s0_, n) in enumerate(WG):
                nsjg = n * J
                gt = nc.gpsimd.tensor_tensor
                r = rw[gi]
                V00g, t3, t2, c3u = Tw[gi]
                C0 = p.tile([128, nsjg], BF16, tag=f"C0{gi}")
                gt(C0[:], V00g[:], r[:], op=MUL)
                C1 = p.tile([128, nsjg], BF16, tag=f"C1{gi}")
                gt(C1[:], t3[:], r[:], op=MUL)
                C2 = p.tile([128, nsjg], BF16, tag=f"C2{gi}")
                gt(C2[:], t2[:], r[:], op=MUL)
                C3 = p.tile([128, nsjg], BF16, tag=f"C3{gi}")
                gt(C3[:], c3u[:], r[:], op=MUL)
                Cw.append((C0, C1, C2, C3))

            def c_slices(l):
                """pieces (n_streams, [c0..c3 APs]) covering layer l's
                streams in order; layer 0 may span several wgroups."""
                res = []
                lo, hi = FO[l], FO[l] + NS[l]
                for gi, (gs, gn) in enumerate(WG):
                    a, b = max(lo, gs), min(hi, gs + gn)
                    if a < b:
                        res.append((b - a, [t[:, (a - gs) * J:(b - gs) * J]
                                            for t in Cw[gi]]))
                return res

            # ---- evaluate the stream tree on the 4 patterns ----
            def eval_piece(l, pi, A, Bv, cs, Hv, on_gpsimd=False):
                nf = A.shape[1]
                c0b, c1b, c2b, c3b = (
                    s.unsqueeze(2).broadcast_to([128, nf, 4]) for s in cs)
                if on_gpsimd:
                    tt = p.tile([128, nf * 4], BF16, tag=f"tt{l}{pi}")
                    tv = tt[:].rearrange("p (m q) -> p m q", q=4)
                    vv = p.tile([128, nf * 4], BF16, tag=f"vv{l}{pi}")
                    vvv = vv[:].rearrange("p (m q) -> p m q", q=4)
                    nc.gpsimd.tensor_tensor(tv, A, Bv, op=MUL)
                    nc.gpsimd.tensor_tensor(vvv, tv, c3b, op=MUL)
                    u1 = p.tile([128, nf * 4], BF16, tag=f"u1{l}{pi}")
                    u1v = u1[:].rearrange("p (m q) -> p m q", q=4)
                    u2 = p.tile([128, nf * 4], BF16, tag=f"u2{l}{pi}")
                    u2v = u2[:].rearrange("p (m q) -> p m q", q=4)
                    nc.vector.tensor_tensor(u1v, A, c1b, op=MUL)
                    nc.vector.tensor_tensor(u2v, Bv, c2b, op=MUL)
                    nc.vector.tensor_tensor(u1v, u1v, u2v, op=ADD)
                    nc.vector.tensor_tensor(u1v, u1v, c0b, op=ADD)
                    nc.vector.tensor_tensor(Hv, u1v, vvv, op=ADD)
                    return
                m1 = p.tile([128, nf * 4], BF16, tag=f"m1{l}{pi}")
                m1v = m1[:].rearrange("p (m q) -> p m q", q=4)
                m4 = p.tile([128, nf * 4], BF16, tag=f"m4{l}{pi}")
                m4v = m4[:].rearrange("p (m q) -> p m q", q=4)
                nc.vector.tensor_tensor(m1v, Bv, c3b, op=MUL)
                nc.vector.tensor_tensor(m1v, m1v, c1b, op=ADD)
                nc.vector.tensor_tensor(m1v, m1v, A, op=MUL)
                nc.vector.tensor_tensor(m4v, Bv, c2b, op=MUL)
                nc.vector.tensor_tensor(m4v, m4v, c0b, op=ADD)
                nc.vector.tensor_tensor(Hv, m1v, m4v, op=ADD)

            Hprev = None
            for l in range(L):
                nf = NS[l] * J
                H = p.tile([128, nf * 4], BF16, tag=f"H{l}")
                Hv = H[:].rearrange("p (m q) -> p m q", q=4)
                if l == 0:
                    A = a0t[:].rearrange("p (m q) -> p m q", q=4)
                    Bv = b0t[:].rearrange("p (m q) -> p m q", q=4)
                else:
                    Hp = Hprev[:].rearrange("p (m q) -> p m q", q=4)
                    A = Hp[:, 0:nf, :]
                    Bv = Hp[:, nf:2 * nf, :]
                pieces = c_slices(l)
                o = 0
                for pi, (nsp, cs) in enumerate(pieces):
                    w = nsp * J
                    eval_piece(l, pi, A[:, o:o + w, :], Bv[:, o:o + w, :],
                               cs, Hv[:, o:o + w, :],
                               on_gpsimd=(l < 2))
                    o += w
                Hprev = H

            # ---- partial GroupSum table -> blend coefficients ----
            # per-partition partial table, converted to multilinear basis
            # BEFORE the broadcast matmul (the basis change is linear)
            H5 = Hprev[:].rearrange("p (j q) -> p j q", q=4)   # [128, J, 4]
            Hred = p.tile([128, 4], F32)
            nc.vector.tensor_reduce(Hred[:], H5.transpose([0, 2, 1]), axis=X, op=ADD)
            gp = p.tile([128, 4], F32)
            up = p.tile([128, 1], F32)
            nc.vector.tensor_copy(gp[:, 0:1], Hred[:, 0:1])
            nc.vector.tensor_tensor(gp[:, 1:2], Hred[:, 1:2], Hred[:, 0:1], op=SUB)
            nc.vector.tensor_tensor(gp[:, 2:3], Hred[:, 2:3], Hred[:, 0:1], op=SUB)
            nc.vector.tensor_tensor(up[:], Hred[:, 3:4], Hred[:, 1:2], op=SUB)
            nc.vector.tensor_tensor(gp[:, 3:4], up[:], gp[:, 2:3], op=SUB)
            ps1 = psp.tile([128, 4], F32)
            nc.tensor.matmul(ps1[:], ones_m[:], gp[:], start=True, stop=True)
            g = p.tile([128, 4], F32)
            nc.scalar.copy(g[:], ps1[:])

            # ---- per-row blend of the full batch ----
            ev = p.tile([128, BROW], F32)
            nc.vector.tensor_scalar(ev[:], s0[:], g[:, 1:2], g[:, 0:1],
                                    op0=MUL, op1=ADD)
            z1 = p.tile([128, BROW], F32)
            nc.vector.scalar_tensor_tensor(z1[:], s1[:], g[:, 2:3], ev[:],
                                           op0=MUL, op1=ADD)
            z2 = p.tile([128, BROW], F32)
            nc.vector.scalar_tensor_tensor(z2[:], t01[:], g[:, 3:4], z1[:],
                                           op0=MUL, op1=ADD)

            osb = p.tile([128, BROW, 2], F32)
            nc.vector.tensor_scalar(osb[:, :, 0], z2[:], ct[:, 0:1], None, op0=MUL)
            nc.vector.tensor_scalar(osb[:, :, 1], z2[:], ct[:, 1:2], None, op0=MUL)
            nc.sync.dma_start(out.rearrange("(p a) c -> p a c", p=128), osb[:])

    nc.compile()
    return nc


def _host_blobs(x, w0, ws, idx0, idxs):
    """Compose the stream tree and build per-core input blobs."""
    x = np.asarray(x, np.float32)
    Wl = [np.asarray(w0, np.float32)] + [np.asarray(ws[i], np.float32)
                                         for i in range(L - 1)]
    Il = [np.asarray(idx0, np.int64)] + [np.asarray(idxs[i], np.int64)
                                         for i in range(L - 1)]

    S = [None] * L
    S[L - 1] = [np.arange(K)]
    for l in range(L - 1, 0, -1):
        S[l - 1] = [Il[l][0][P] for P in S[l]] + [Il[l][1][P] for P in S[l]]

    # wall: [cores, 128, (l,s), J, 16]
    wall = np.empty((N_CORES, 128, NSTOT, J, 16), np.float32)
    for l in range(L):
        for s in range(NS[l]):
            pw = Wl[l][S[l][s]]                       # [K, 16]
            pw = pw.reshape(N_CORES, J, 128, 16)      # core, j, p, i
            wall[:, :, FO[l] + s, :, :] = pw.transpose(0, 2, 1, 3)

    # layer-0 pattern inputs: a0[core, p, s, j, q] = (q >> m0) & 1
    q = np.arange(4)
    a0 = np.empty((N_CORES, 128, NS[0], J, 4), np.float32)
    b0 = np.empty((N_CORES, 128, NS[0], J, 4), np.float32)
    for s in range(NS[0]):
        m0 = Il[0][0][S[0][s]].reshape(N_CORES, J, 128)  # core, j, p
        m1 = Il[0][1][S[0][s]].reshape(N_CORES, J, 128)
        a0[:, :, s, :, :] = ((q[None, None, None, :] >> m0.transpose(0, 2, 1)[..., None]) & 1)
        b0[:, :, s, :, :] = ((q[None, None, None, :] >> m1.transpose(0, 2, 1)[..., None]) & 1)
    a0 = a0.reshape(N_CORES, 128, NS[0] * J * 4)
    b0 = b0.reshape(N_CORES, 128, NS[0] * J * 4)

    xin = np.ascontiguousarray(x.reshape(128, BROW, 2))
    in_maps = []
    for ci in range(N_CORES):
        cls = np.array([1.0, 0.0] if ci < N_CORES // 2 else [0.0, 1.0], np.float32)
        m = {
            "a0in": np.ascontiguousarray(a0[ci]).astype(ml_dtypes.bfloat16),
            "b0in": np.ascontiguousarray(b0[ci]).astype(ml_dtypes.bfloat16),
            "xin": xin,
            "clsg": np.tile(cls, (128, 1)),
        }
        for gi, (gs, gn) in enumerate(WG):
            m[f"wall{gi}"] = np.ascontiguousarray(
                wall[ci, :, gs:gs + gn, :, :].reshape(128, -1)).astype(
                    ml_dtypes.bfloat16)
        in_maps.append(m)
    return in_maps


def run(inputs, trace=False, trace_kwargs=None):
    global _compiled
    if _compiled is None:
        _compiled = _build_program()
    nc = _compiled
    in_maps = _host_blobs(inputs["x"], inputs["w0"], inputs["ws"],
                          inputs["idx0"], inputs["idxs"])
    res = run_bass_kernel_spmd(nc, in_maps, core_ids=list(range(N_CORES)),
                               trace=trace, **(trace_kwargs or {}))
    total = np.zeros((B, 2), np.float32)
    for ci in range(N_CORES):
        total += res.results[ci]["out"]
    return total, res


def kernel(x, w0, ws, idx0, idxs):
    out, _ = run({"x": x, "w0": w0, "ws": ws, "idx0": idx0, "idxs": idxs})
    return out



# revision 8
# speedup vs baseline: 1.1126x; 1.1126x over previous
"""Trainium2 Bass kernel for nn_DiffLogicPBF (difflogic network).

Algorithm
---------
The network input is binarized to 2 bits, so every batch row's activation
trajectory takes one of only 4 "patterns".  We evaluate the network on the 4
patterns instead of 8192 rows, then blend per-row.

The per-layer gathers are composed on the host into a stream tree (layer l is
evaluated 2^(5-l) times in permuted orders, 63 streams total), so the device
does gather-free elementwise work only.  Weights arrive pre-permuted.

Device pipeline per core (512 neurons x 63 streams = 252 columns of 128):
  exp(w) on ACT  ->  PE matmul with a constant 16->5 matrix that computes the
  multilinear coefficients c0..c3 and the softmax denominator D per neuron
  (one matmul per 1024-neuron chunk, E as the stationary operand so the
  output lands neuron-major)  ->  PSUM->SBUF copy into per-coefficient dense
  planes  ->  elementwise multilinear evaluation over the 4 patterns on
  DVE+GpSimd.  Divisions are folded into the next layer's coefficients
  (c*1/D of the producing column), so the eval critical path is 4 ops/layer.

Sharding: neurons are split across the 8 cores (512 each).  Each core builds
its partial GroupSum table, blends the full batch against it, and the host
sums the 8 partial [B,2] outputs.
"""

from contextlib import ExitStack

import ml_dtypes
import numpy as np

import concourse.bacc as bacc
import concourse.mybir as mybir
import concourse.tile as tile
from concourse.bass_utils import run_bass_kernel_spmd

F32 = mybir.dt.float32
BF16 = mybir.dt.bfloat16
ADD = mybir.AluOpType.add
SUB = mybir.AluOpType.subtract
MUL = mybir.AluOpType.mult
GT = mybir.AluOpType.is_gt
X = mybir.AxisListType.X
EXP = mybir.ActivationFunctionType.Exp

N_CORES = 8
B, K, L = 8192, 4096, 6
NS = [32, 16, 8, 4, 2, 1]            # streams per layer
FO = np.cumsum([0] + NS).tolist()    # stream offsets by layer
COLB = [f * 4 for f in FO]           # column base per layer: [0,128,192,224,240,248,252]
NCOL = 252
NCH = 32                             # 8-column chunks (incl. half-chunk of pad)
BROW = B // 128

_compiled = None


def _build_program():
    nc = bacc.Bacc("TRN2", target_bir_lowering=False, debug=False,
                   num_devices=N_CORES)
    wallin = nc.dram_tensor("wallin", [128, 4096], BF16, kind="ExternalInput").ap()
    abin = nc.dram_tensor("abin", [128, 1064], BF16, kind="ExternalInput").ap()
    xcin = nc.dram_tensor("xcin", [128, 258], F32, kind="ExternalInput").ap()
    out = nc.dram_tensor("out", [B, 2], F32, kind="ExternalOutput").ap()

    with tile.TileContext(nc) as tc:
        with ExitStack() as ctx:
            p = ctx.enter_context(tc.tile_pool(name="p", bufs=1))
            psp = ctx.enter_context(tc.tile_pool(name="ps", bufs=1, space="PSUM"))

            # ---- input DMAs ----
            wall = p.tile([128, 4096], BF16)
            nc.sync.dma_start(wall[:, 0:2048], wallin[:, 0:2048])
            nc.sync.dma_start(wall[:, 2048:4096], wallin[:, 2048:4096])
            ab = p.tile([128, 1064], BF16)
            nc.gpsimd.dma_start(ab[:], abin[:])
            xc = p.tile([128, 258], F32)
            nc.gpsimd.dma_start(xc[:], xcin[:])

            av = ab[:, 0:512].rearrange("p (q c) -> p q c", c=128)
            bv = ab[:, 512:1024].rearrange("p (q c) -> p q c", c=128)
            kmv = ab[:, 1024:1064]
            ones = xc[:, 130:258]

            # PE warmup (starts the pstate ramp); keep-alive via gp copy back
            # into the ones block (value is exactly 1.0, so a no-op).
            wm = psp.tile([1, 1], F32)
            nc.tensor.matmul(wm[:], ones[0:1, 0:1], ones[0:1, 0:1],
                             start=True, stop=True)
            nc.scalar.copy(ones[0:1, 0:1], wm[:])

            # blend prep on DVE while weights are in flight
            xv = xc[:, 0:128].rearrange("p (a c) -> p a c", c=2)
            s0 = p.tile([128, BROW], F32)
            nc.vector.tensor_scalar(s0[:], xv[:, :, 0], 0.0, None, op0=GT)
            s1 = p.tile([128, BROW], F32)
            nc.vector.tensor_scalar(s1[:], xv[:, :, 1], 0.0, None, op0=GT)
            t01 = p.tile([128, BROW], F32)
            nc.vector.tensor_tensor(t01[:], s0[:], s1[:], op=MUL)

            # ---- exp on ACT, 4 chunks ----
            E = p.tile([128, 4096], BF16)
            for k in range(4):
                nc.scalar.activation(E[:, k * 1024:(k + 1) * 1024],
                                     wall[:, k * 1024:(k + 1) * 1024], EXP)

            # ---- coefficient matmuls: one per 8-column chunk ----
            psb = [psp.tile([128, 320], F32, tag=f"pb{b}", name=f"pb{b}")
                   for b in range(4)]
            for c in range(NCH):
                b, s = c // 8, c % 8
                nc.tensor.matmul(psb[b][:, s * 40:(s + 1) * 40],
                                 E[:, c * 128:(c + 1) * 128], kmv,
                                 start=True, stop=True)

            # ---- PSUM -> SBUF k-plane copies + reciprocals ----
            # slabK[p, k, col]: dense per-coefficient planes (k=4 holds D)
            slabK = p.tile([128, 5, 256], BF16)
            rall = p.tile([128, 256], F32)

            def copy_bank(b, eng):
                inv = psb[b][:].rearrange("p (s k g) -> p k s g", k=5, g=8)
                outv = slabK[:, :, b * 64:(b + 1) * 64].rearrange(
                    "p k (s g) -> p k s g", g=8)
                eng.tensor_copy(outv[:, 0:4], inv[:, 0:4])

            def recip_bank(b):
                dv = psb[b][:].rearrange("p (s k g) -> p k s g", k=5, g=8)
                nc.vector.reciprocal(
                    rall[:, b * 64:(b + 1) * 64].rearrange("p (s g) -> p s g", g=8),
                    dv[:, 4])

            # folded coefficients for layers 1..5 (on gpsimd, off critical path)
            ch = {}

            def chat(l):
                lo, hi = COLB[l], COLB[l + 1]
                n = hi - lo
                plo = COLB[l - 1]
                rA = rall[:, plo:plo + n]
                rB = rall[:, plo + n:plo + 2 * n]
                c1 = p.tile([128, n], BF16, tag=f"c1h{l}")
                nc.gpsimd.tensor_tensor(c1[:], slabK[:, 1, lo:hi], rA, op=MUL)
                c2 = p.tile([128, n], BF16, tag=f"c2h{l}")
                nc.gpsimd.tensor_tensor(c2[:], slabK[:, 2, lo:hi], rB, op=MUL)
                c3t = p.tile([128, n], BF16, tag=f"c3t{l}")
                nc.gpsimd.tensor_tensor(c3t[:], slabK[:, 3, lo:hi], rA, op=MUL)
                c3 = p.tile([128, n], BF16, tag=f"c3h{l}")
                nc.gpsimd.tensor_tensor(c3[:], c3t[:], rB, op=MUL)
                ch[l] = (c1, c2, c3)

            H = {}

            def bc(apl, n):
                return apl.unsqueeze(1).broadcast_to([128, 4, n])

            def eval_piece(l, lo, hi, tag):
                n = hi - lo
                if l == 0:
                    A = av[:, :, lo:hi]
                    Bv = bv[:, :, lo:hi]
                    c1b = bc(slabK[:, 1, lo:hi], n)
                    c2b = bc(slabK[:, 2, lo:hi], n)
                    c3b = bc(slabK[:, 3, lo:hi], n)
                else:
                    Hp = H[l - 1]
                    A = Hp[:, :, 0:n]
                    Bv = Hp[:, :, n:2 * n]
                    c1t, c2t, c3t_ = ch[l]
                    c1b = bc(c1t[:], n)
                    c2b = bc(c2t[:], n)
                    c3b = bc(c3t_[:], n)
                c0b = bc(slabK[:, 0, lo:hi], n)
                llo = lo - COLB[l]
                Hv = H[l][:, :, llo:llo + n]
                t = p.tile([128, 4, n], BF16, tag=f"t{tag}")
                nc.vector.tensor_tensor(t[:], Bv, c3b, op=MUL)
                nc.vector.tensor_tensor(t[:], t[:], c1b, op=ADD)
                m = p.tile([128, 4, n], BF16, tag=f"m{tag}")
                nc.vector.tensor_tensor(m[:], t[:], A, op=MUL)
                u = p.tile([128, 4, n], BF16, tag=f"u{tag}")
                nc.gpsimd.tensor_tensor(u[:], Bv, c2b, op=MUL)
                nc.gpsimd.tensor_tensor(u[:], u[:], c0b, op=ADD)
                nc.vector.tensor_tensor(Hv, m[:], u[:], op=ADD)

            for l in range(L):
                n = COLB[l + 1] - COLB[l]
                H[l] = p.tile([128, 4, n], BF16, tag=f"H{l}", name=f"H{l}")

            # ---- interleaved schedule ----
            copy_bank(0, nc.vector)
            recip_bank(0)
            eval_piece(0, 0, 64, "l0a")
            copy_bank(1, nc.vector)
            recip_bank(1)
            eval_piece(0, 64, 128, "l0b")
            copy_bank(2, nc.vector)
            recip_bank(2)
            chat(1)
            eval_piece(1, 128, 192, "l1")
            copy_bank(3, nc.vector)
            recip_bank(3)
            chat(2)
            eval_piece(2, 192, 224, "l2")
            chat(3)
            eval_piece(3, 224, 240, "l3")
            chat(4)
            eval_piece(4, 240, 248, "l4")
            chat(5)
            eval_piece(5, 248, 252, "l5")

            # ---- partial GroupSum table ----
            r5b = rall[:, 248:252].unsqueeze(1).broadcast_to([128, 4, 4])
            M = p.tile([128, 4, 4], F32)
            nc.vector.tensor_tensor(M[:], H[5][:], r5b, op=MUL)
            Hred = p.tile([128, 4], F32)
            nc.vector.tensor_reduce(Hred[:], M[:], axis=X, op=ADD)

            gpt = p.tile([128, 4], F32)
            up = p.tile([128, 1], F32)
            nc.vector.tensor_copy(gpt[:, 0:1], Hred[:, 0:1])
            nc.vector.tensor_tensor(gpt[:, 1:2], Hred[:, 1:2], Hred[:, 0:1], op=SUB)
            nc.vector.tensor_tensor(gpt[:, 2:3], Hred[:, 2:3], Hred[:, 0:1], op=SUB)
            nc.vector.tensor_tensor(up[:], Hred[:, 3:4], Hred[:, 1:2], op=SUB)
            nc.vector.tensor_tensor(gpt[:, 3:4], up[:], gpt[:, 2:3], op=SUB)

            psg = psp.tile([128, 4], F32)
            nc.tensor.matmul(psg[:], ones[:], gpt[:], start=True, stop=True)
            g = p.tile([128, 4], F32)
            nc.scalar.copy(g[:], psg[:])

            # ---- per-row blend of the full batch ----
            ev = p.tile([128, BROW], F32)
            nc.vector.tensor_scalar(ev[:], s0[:], g[:, 1:2], g[:, 0:1],
                                    op0=MUL, op1=ADD)
            z1 = p.tile([128, BROW], F32)
            nc.vector.scalar_tensor_tensor(z1[:], s1[:], g[:, 2:3], ev[:],
                                           op0=MUL, op1=ADD)
            z2 = p.tile([128, BROW], F32)
            nc.vector.scalar_tensor_tensor(z2[:], t01[:], g[:, 3:4], z1[:],
                                           op0=MUL, op1=ADD)

            osb = p.tile([128, BROW, 2], F32)
            nc.vector.tensor_scalar(osb[:, :, 0], z2[:], xc[:, 128:129], None, op0=MUL)
            nc.vector.tensor_scalar(osb[:, :, 1], z2[:], xc[:, 129:130], None, op0=MUL)
            nc.sync.dma_start(out.rearrange("(p a) c -> p a c", p=128), osb[:])

    nc.compile()
    return nc


def _host_blobs(x, w0, ws, idx0, idxs):
    """Compose the stream tree and build per-core input blobs."""
    x = np.asarray(x, np.float32)
    Wl = [np.asarray(w0, np.float32)] + [np.asarray(ws[i], np.float32)
                                         for i in range(L - 1)]
    Il = [np.asarray(idx0, np.int64)] + [np.asarray(idxs[i], np.int64)
                                         for i in range(L - 1)]

    S = [None] * L
    S[L - 1] = [np.arange(K)]
    for l in range(L - 1, 0, -1):
        S[l - 1] = [Il[l][0][P] for P in S[l]] + [Il[l][1][P] for P in S[l]]

    # weights in column order: wtmp[core, col, p, i], col = 4*stream + j
    wtmp = np.zeros((N_CORES, 256, 128, 16), np.float32)
    for l in range(L):
        for s in range(NS[l]):
            sg = FO[l] + s
            pw = Wl[l][S[l][s]].reshape(N_CORES, 4, 128, 16)
            wtmp[:, sg * 4:(sg + 1) * 4] = pw
    # wall[core, g*16+i, c*128+p], col = c*8+g
    wt = wtmp.reshape(N_CORES, 32, 8, 128, 16)
    wall = np.ascontiguousarray(
        wt.transpose(0, 2, 4, 1, 3).reshape(N_CORES, 128, 4096))

    # layer-0 pattern inputs, pattern-major: a0[core, p, q*128 + col]
    q = np.arange(4)
    msel0 = np.zeros((N_CORES, 128, 128), np.int64)  # [core, col, p]
    msel1 = np.zeros((N_CORES, 128, 128), np.int64)
    for s in range(NS[0]):
        idx = S[0][s].reshape(N_CORES, 4, 128)
        msel0[:, s * 4:(s + 1) * 4] = Il[0][0][idx]
        msel1[:, s * 4:(s + 1) * 4] = Il[0][1][idx]
    a0 = (q[None, :, None, None] >> msel0[:, None, :, :]) & 1   # [core,q,col,p]
    b0 = (q[None, :, None, None] >> msel1[:, None, :, :]) & 1
    a0 = a0.transpose(0, 3, 1, 2).reshape(N_CORES, 128, 512)    # [core,p,(q,col)]
    b0 = b0.transpose(0, 3, 1, 2).reshape(N_CORES, 128, 512)

    # constant 16->5 coefficient matrix, block-diagonal over 8 groups
    i16 = np.arange(16)
    t11, t10 = i16 & 1, (i16 >> 1) & 1
    t01, t00 = (i16 >> 2) & 1, (i16 >> 3) & 1
    KC = np.stack([t00, t10 - t00, t01 - t00,
                   t11 - t10 - t01 + t00, np.ones(16, np.int64)], 1)  # [16,5]
    kb = np.zeros((8, 16, 5, 8), np.float32)
    for gidx in range(8):
        kb[gidx, :, :, gidx] = KC
    kblob = kb.reshape(128, 40)

    xpart = np.ascontiguousarray(x.reshape(128, 128))
    ones = np.ones((128, 128), np.float32)

    in_maps = []
    for ci in range(N_CORES):
        cls = np.array([1.0, 0.0] if ci < N_CORES // 2 else [0.0, 1.0],
                       np.float32)
        abm = np.concatenate(
            [a0[ci], b0[ci], kblob], axis=1).astype(ml_dtypes.bfloat16)
        xcm = np.concatenate(
            [xpart, np.tile(cls, (128, 1)), ones], axis=1).astype(np.float32)
        in_maps.append({
            "wallin": wall[ci].astype(ml_dtypes.bfloat16),
            "abin": np.ascontiguousarray(abm),
            "xcin": np.ascontiguousarray(xcm),
        })
    return in_maps


def run(inputs, trace=False, trace_kwargs=None):
    global _compiled
    if _compiled is None:
        _compiled = _build_program()
    nc = _compiled
    in_maps = _host_blobs(inputs["x"], inputs["w0"], inputs["ws"],
                          inputs["idx0"], inputs["idxs"])
    res = run_bass_kernel_spmd(nc, in_maps, core_ids=list(range(N_CORES)),
                               trace=trace, **(trace_kwargs or {}))
    total = np.zeros((B, 2), np.float32)
    for ci in range(N_CORES):
        total += res.results[ci]["out"]
    return total, res


def kernel(x, w0, ws, idx0, idxs):
    out, _ = run({"x": x, "w0": w0, "ws": ws, "idx0": idx0, "idxs": idxs})
    return out


# revision 10
# speedup vs baseline: 1.3257x; 1.1915x over previous
"""Trainium2 Bass kernel for nn_DiffLogicPBF (difflogic network).

Algorithm
---------
The network input is binarized to 2 bits, so every batch row's activation
trajectory takes one of only 4 "patterns".  We evaluate the network on the 4
patterns instead of 8192 rows, then blend per-row.

The per-layer gathers are composed on the host into a stream tree (layer l is
evaluated 2^(5-l) times in permuted orders, 63 streams total), so the device
does gather-free elementwise work only.  Weights arrive pre-permuted, in fp8
(the softmax input tolerates ~3% quantization; the batch-summed output error
stays ~1e-3).

Device pipeline per core (512 neurons x 63 streams = 252 columns of 128):
  exp(w) on ACT  ->  PE matmul with a constant 16->5 matrix that computes the
  multilinear coefficients c0..c3 and the softmax denominator D per neuron
  (one matmul per 1024-neuron chunk, E as the stationary operand so the
  output lands neuron-major)  ->  PSUM->SBUF copy into per-coefficient dense
  planes  ->  elementwise multilinear evaluation over the 4 patterns on
  DVE+GpSimd.  Divisions are folded into the next layer's coefficients
  (c * 1/D of the producing column), keeping the eval critical path at ~4
  dependent ops per layer.

Sharding: neurons are split across the 8 cores (512 each).  Each core builds
its partial GroupSum table, blends the full batch against it, and the host
sums the 8 partial [B,2] outputs.
"""

from contextlib import ExitStack

import ml_dtypes
import numpy as np

import concourse.bacc as bacc
import concourse.mybir as mybir
import concourse.tile as tile
from concourse.bass_utils import run_bass_kernel_spmd

F32 = mybir.dt.float32
BF16 = mybir.dt.bfloat16
FP8 = mybir.dt.float8e4
ADD = mybir.AluOpType.add
SUB = mybir.AluOpType.subtract
MUL = mybir.AluOpType.mult
GT = mybir.AluOpType.is_gt
X = mybir.AxisListType.X
EXP = mybir.ActivationFunctionType.Exp

N_CORES = 8
B, K, L = 8192, 4096, 6
NS = [32, 16, 8, 4, 2, 1]            # streams per layer
FO = np.cumsum([0] + NS).tolist()    # stream offsets by layer
COLB = [f * 4 for f in FO]           # column base per layer
NCH = 32                             # 8-column chunks (incl. half-chunk of pad)
BROW = B // 128

_compiled = None


def _build_program():
    nc = bacc.Bacc("TRN2", target_bir_lowering=False, debug=False,
                   num_devices=N_CORES)
    wallin = nc.dram_tensor("wallin", [128, 4096], FP8, kind="ExternalInput").ap()
    abin = nc.dram_tensor("abin", [128, 1024], FP8, kind="ExternalInput").ap()
    xkin = nc.dram_tensor("xkin", [128, 170], BF16, kind="ExternalInput").ap()
    out = nc.dram_tensor("out", [B, 2], F32, kind="ExternalOutput").ap()

    with tile.TileContext(nc) as tc:
        with ExitStack() as ctx:
            p = ctx.enter_context(tc.tile_pool(name="p", bufs=1))
            psp = ctx.enter_context(tc.tile_pool(name="ps", bufs=1, space="PSUM"))

            # ---- input DMAs (xk first: kmat gates every matmul) ----
            xk = p.tile([128, 170], BF16)
            nc.sync.dma_start(xk[:], xkin[:])
            wall = p.tile([128, 4096], FP8)
            nc.sync.dma_start(wall[:, 0:1024], wallin[:, 0:1024])
            nc.sync.dma_start(wall[:, 1024:4096], wallin[:, 1024:4096])
            ab = p.tile([128, 1024], FP8)
            nc.gpsimd.dma_start(ab[:], abin[:])

            kmv = xk[:, 130:170]

            # ones for the table-broadcast matmul, built on device
            onesb = p.tile([128, 128], BF16)
            nc.gpsimd.memset(onesb[:], 1.0)

            # blend prep on DVE while weights are in flight
            xv = xk[:, 0:128].rearrange("p (a c) -> p a c", c=2)
            s0 = p.tile([128, BROW], F32)
            nc.vector.tensor_scalar(s0[:], xv[:, :, 0], 0.0, None, op0=GT)
            s1 = p.tile([128, BROW], F32)
            nc.vector.tensor_scalar(s1[:], xv[:, :, 1], 0.0, None, op0=GT)
            t01 = p.tile([128, BROW], F32)
            nc.vector.tensor_tensor(t01[:], s0[:], s1[:], op=MUL)

            # layer-0 pattern inputs, cast to bf16 once (keeps eval in 2x mode)
            av = p.tile([128, 4, 128], BF16)
            nc.vector.tensor_copy(av[:], ab[:, 0:512].rearrange(
                "p (q c) -> p q c", c=128))
            bv = p.tile([128, 4, 128], BF16)
            nc.gpsimd.tensor_copy(bv[:], ab[:, 512:1024].rearrange(
                "p (q c) -> p q c", c=128))

            # ---- exp on ACT, 4 chunks ----
            E = p.tile([128, 4096], BF16)
            for k in range(4):
                nc.scalar.activation(E[:, k * 1024:(k + 1) * 1024],
                                     wall[:, k * 1024:(k + 1) * 1024], EXP)

            # ---- coefficient matmuls: one per 8-column chunk ----
            psb = [psp.tile([128, 320], F32, tag=f"pb{b}", name=f"pb{b}")
                   for b in range(4)]
            for c in range(NCH):
                b, s = c // 8, c % 8
                nc.tensor.matmul(psb[b][:, s * 40:(s + 1) * 40],
                                 E[:, c * 128:(c + 1) * 128], kmv,
                                 start=True, stop=True)

            # ---- PSUM -> SBUF k-plane copies + reciprocals ----
            # slabK[p, k, col]: dense per-coefficient planes
            slabK = p.tile([128, 5, 256], BF16)
            rall = p.tile([128, 256], F32)

            def copy_bank(b):
                inv = psb[b][:].rearrange("p (s k g) -> p k s g", k=5, g=8)
                outv = slabK[:, :, b * 64:(b + 1) * 64].rearrange(
                    "p k (s g) -> p k s g", g=8)
                nc.vector.tensor_copy(outv[:, 0:4], inv[:, 0:4])

            def recip_bank(b):
                dv = psb[b][:].rearrange("p (s k g) -> p k s g", k=5, g=8)
                nc.vector.reciprocal(
                    rall[:, b * 64:(b + 1) * 64].rearrange("p (s g) -> p s g", g=8),
                    dv[:, 4])

            # folded coefficients for layers 1..5 (gpsimd, off critical path)
            ch = {}

            def chat(l):
                lo, hi = COLB[l], COLB[l + 1]
                n = hi - lo
                plo = COLB[l - 1]
                rA = rall[:, plo:plo + n]
                rB = rall[:, plo + n:plo + 2 * n]
                c1 = p.tile([128, n], BF16, tag=f"c1h{l}", name=f"c1h{l}")
                nc.gpsimd.tensor_tensor(c1[:], slabK[:, 1, lo:hi], rA, op=MUL)
                c2 = p.tile([128, n], BF16, tag=f"c2h{l}", name=f"c2h{l}")
                nc.gpsimd.tensor_tensor(c2[:], slabK[:, 2, lo:hi], rB, op=MUL)
                c3t = p.tile([128, n], BF16, tag=f"c3t{l}", name=f"c3t{l}")
                nc.gpsimd.tensor_tensor(c3t[:], slabK[:, 3, lo:hi], rA, op=MUL)
                c3 = p.tile([128, n], BF16, tag=f"c3h{l}", name=f"c3h{l}")
                nc.gpsimd.tensor_tensor(c3[:], c3t[:], rB, op=MUL)
                ch[l] = (c1, c2, c3)

            H = {}
            for l in range(L):
                n = COLB[l + 1] - COLB[l]
                H[l] = p.tile([128, 4, n], BF16, tag=f"H{l}", name=f"H{l}")

            def bc(apl, n):
                return apl.unsqueeze(1).broadcast_to([128, 4, n])

            def eval_piece(l, lo, hi, tag, u_eng):
                n = hi - lo
                if l == 0:
                    A = av[:, :, lo:hi]
                    Bv = bv[:, :, lo:hi]
                    c1b = bc(slabK[:, 1, lo:hi], n)
                    c2b = bc(slabK[:, 2, lo:hi], n)
                    c3b = bc(slabK[:, 3, lo:hi], n)
                else:
                    Hp = H[l - 1]
                    A = Hp[:, :, 0:n]
                    Bv = Hp[:, :, n:2 * n]
                    c1t, c2t, c3t_ = ch[l]
                    c1b = bc(c1t[:], n)
                    c2b = bc(c2t[:], n)
                    c3b = bc(c3t_[:], n)
                c0b = bc(slabK[:, 0, lo:hi], n)
                llo = lo - COLB[l]
                Hv = H[l][:, :, llo:llo + n]
                t = p.tile([128, 4, n], BF16, tag=f"t{tag}", name=f"t{tag}")
                u = p.tile([128, 4, n], BF16, tag=f"u{tag}", name=f"u{tag}")
                nc.vector.tensor_tensor(t[:], Bv, c3b, op=MUL)
                u_eng.tensor_tensor(u[:], Bv, c2b, op=MUL)
                nc.vector.tensor_tensor(t[:], t[:], c1b, op=ADD)
                u_eng.tensor_tensor(u[:], u[:], c0b, op=ADD)
                m = p.tile([128, 4, n], BF16, tag=f"m{tag}", name=f"m{tag}")
                nc.vector.tensor_tensor(m[:], t[:], A, op=MUL)
                nc.vector.tensor_tensor(Hv, m[:], u[:], op=ADD)

            # ---- interleaved schedule ----
            copy_bank(0)
            eval_piece(0, 0, 64, "l0a", nc.gpsimd)
            recip_bank(0)
            copy_bank(1)
            eval_piece(0, 64, 128, "l0b", nc.gpsimd)
            recip_bank(1)
            copy_bank(2)
            recip_bank(2)
            chat(1)
            eval_piece(1, 128, 192, "l1", nc.vector)
            copy_bank(3)
            recip_bank(3)
            chat(2)
            eval_piece(2, 192, 224, "l2", nc.vector)
            chat(3)
            eval_piece(3, 224, 240, "l3", nc.vector)
            chat(4)
            eval_piece(4, 240, 248, "l4", nc.vector)
            chat(5)
            eval_piece(5, 248, 252, "l5", nc.vector)

            # ---- partial GroupSum table ----
            r5b = rall[:, 248:252].unsqueeze(1).broadcast_to([128, 4, 4])
            M = p.tile([128, 4, 4], F32)
            nc.vector.tensor_tensor(M[:], H[5][:], r5b, op=MUL)
            Hred = p.tile([128, 4], F32)
            nc.vector.tensor_reduce(Hred[:], M[:], axis=X, op=ADD)

            gpt = p.tile([128, 4], BF16)
            up = p.tile([128, 1], F32)
            nc.vector.tensor_copy(gpt[:, 0:1], Hred[:, 0:1])
            nc.vector.tensor_tensor(gpt[:, 1:2], Hred[:, 1:2], Hred[:, 0:1], op=SUB)
            nc.vector.tensor_tensor(gpt[:, 2:3], Hred[:, 2:3], Hred[:, 0:1], op=SUB)
            nc.vector.tensor_tensor(up[:], Hred[:, 3:4], Hred[:, 1:2], op=SUB)
            nc.vector.tensor_tensor(gpt[:, 3:4], up[:], gpt[:, 2:3], op=SUB)

            psg = psp.tile([128, 4], F32)
            nc.tensor.matmul(psg[:], onesb[:], gpt[:], start=True, stop=True)
            g = p.tile([128, 4], F32)
            nc.scalar.copy(g[:], psg[:])

            # ---- per-row blend of the full batch ----
            ev = p.tile([128, BROW], F32)
            nc.vector.tensor_scalar(ev[:], s0[:], g[:, 1:2], g[:, 0:1],
                                    op0=MUL, op1=ADD)
            z1 = p.tile([128, BROW], F32)
            nc.vector.scalar_tensor_tensor(z1[:], s1[:], g[:, 2:3], ev[:],
                                           op0=MUL, op1=ADD)
            z2 = p.tile([128, BROW], F32)
            nc.vector.scalar_tensor_tensor(z2[:], t01[:], g[:, 3:4], z1[:],
                                           op0=MUL, op1=ADD)

            osb = p.tile([128, BROW, 2], F32)
            nc.vector.tensor_tensor(osb[:, :, 0], z2[:],
                                    xk[:, 128:129].broadcast_to([128, BROW]), op=MUL)
            nc.vector.tensor_tensor(osb[:, :, 1], z2[:],
                                    xk[:, 129:130].broadcast_to([128, BROW]), op=MUL)
            nc.sync.dma_start(out.rearrange("(p a) c -> p a c", p=128), osb[:])

    nc.compile()
    return nc


def _host_blobs(x, w0, ws, idx0, idxs):
    """Compose the stream tree and build per-core input blobs."""
    x = np.asarray(x, np.float32)
    Wl = [np.asarray(w0, np.float32)] + [np.asarray(ws[i], np.float32)
                                         for i in range(L - 1)]
    Il = [np.asarray(idx0, np.int64)] + [np.asarray(idxs[i], np.int64)
                                         for i in range(L - 1)]

    S = [None] * L
    S[L - 1] = [np.arange(K)]
    for l in range(L - 1, 0, -1):
        S[l - 1] = [Il[l][0][P] for P in S[l]] + [Il[l][1][P] for P in S[l]]

    # weights in column order: wtmp[core, col, p, i], col = 4*stream + j
    wtmp = np.zeros((N_CORES, 256, 128, 16), np.float32)
    for l in range(L):
        for s in range(NS[l]):
            sg = FO[l] + s
            pw = Wl[l][S[l][s]].reshape(N_CORES, 4, 128, 16)
            wtmp[:, sg * 4:(sg + 1) * 4] = pw
    # wall[core, g*16+i, c*128+p], col = c*8+g
    wt = wtmp.reshape(N_CORES, 32, 8, 128, 16)
    wall = np.ascontiguousarray(
        wt.transpose(0, 2, 4, 1, 3).reshape(N_CORES, 128, 4096))

    # layer-0 pattern inputs, pattern-major: a0[core, p, q*128 + col]
    q = np.arange(4)
    msel0 = np.zeros((N_CORES, 128, 128), np.int64)  # [core, col, p]
    msel1 = np.zeros((N_CORES, 128, 128), np.int64)
    for s in range(NS[0]):
        idx = S[0][s].reshape(N_CORES, 4, 128)
        msel0[:, s * 4:(s + 1) * 4] = Il[0][0][idx]
        msel1[:, s * 4:(s + 1) * 4] = Il[0][1][idx]
    a0 = (q[None, :, None, None] >> msel0[:, None, :, :]) & 1   # [core,q,col,p]
    b0 = (q[None, :, None, None] >> msel1[:, None, :, :]) & 1
    a0 = a0.transpose(0, 3, 1, 2).reshape(N_CORES, 128, 512)    # [core,p,(q,col)]
    b0 = b0.transpose(0, 3, 1, 2).reshape(N_CORES, 128, 512)

    # constant 16->5 coefficient matrix, block-diagonal over 8 groups
    i16 = np.arange(16)
    t11, t10 = i16 & 1, (i16 >> 1) & 1
    t01, t00 = (i16 >> 2) & 1, (i16 >> 3) & 1
    KC = np.stack([t00, t10 - t00, t01 - t00,
                   t11 - t10 - t01 + t00, np.ones(16, np.int64)], 1)  # [16,5]
    kb = np.zeros((8, 16, 5, 8), np.float32)
    for gidx in range(8):
        kb[gidx, :, :, gidx] = KC
    kblob = kb.reshape(128, 40)

    xpart = np.ascontiguousarray(x.reshape(128, 128))

    in_maps = []
    for ci in range(N_CORES):
        cls = np.array([1.0, 0.0] if ci < N_CORES // 2 else [0.0, 1.0],
                       np.float32)
        abm = np.concatenate([a0[ci], b0[ci]], axis=1)
        xkm = np.concatenate(
            [xpart, np.tile(cls, (128, 1)), kblob], axis=1)
        in_maps.append({
            "wallin": wall[ci].astype(ml_dtypes.float8_e4m3fn),
            "abin": np.ascontiguousarray(abm).astype(ml_dtypes.float8_e4m3fn),
            "xkin": np.ascontiguousarray(xkm).astype(ml_dtypes.bfloat16),
        })
    return in_maps


def run(inputs, trace=False, trace_kwargs=None):
    global _compiled
    if _compiled is None:
        _compiled = _build_program()
    nc = _compiled
    in_maps = _host_blobs(inputs["x"], inputs["w0"], inputs["ws"],
                          inputs["idx0"], inputs["idxs"])
    res = run_bass_kernel_spmd(nc, in_maps, core_ids=list(range(N_CORES)),
                               trace=trace, **(trace_kwargs or {}))
    total = np.zeros((B, 2), np.float32)
    for ci in range(N_CORES):
        total += res.results[ci]["out"]
    return total, res


def kernel(x, w0, ws, idx0, idxs):
    out, _ = run({"x": x, "w0": w0, "ws": ws, "idx0": idx0, "idxs": idxs})
    return out


# revision 11
# speedup vs baseline: 1.3312x; 1.0042x over previous
"""Trainium2 Bass kernel for nn_DiffLogicPBF (difflogic network).

Algorithm
---------
The network input is binarized to 2 bits, so every batch row's activation
trajectory takes one of only 4 "patterns".  We evaluate the network on the 4
patterns instead of 8192 rows, then blend per-row.

The per-layer gathers are composed on the host into a stream tree (layer l is
evaluated 2^(5-l) times in permuted orders, 63 streams total), so the device
does gather-free elementwise work only.  Weights arrive pre-permuted, in fp8.

Device pipeline per core (512 neurons x 63 streams = 252 columns of 128):
  exp(w) on ACT  ->  PE matmul with a constant 16->5 matrix that computes the
  multilinear coefficients c0..c3 and the softmax denominator D per neuron
  (one matmul per 1024-neuron chunk, E as the stationary operand so the
  output lands neuron-major)  ->  r = 1/D per column, then one fused
  PSUM->SBUF multiply that both converts and NORMALIZES the coefficients
  (slab = psum * r), so the eval needs no divisions at all  ->  elementwise
  multilinear evaluation over the 4 patterns, 6 tensor_tensor ops per layer.

Sharding: neurons are split across the 8 cores (512 each).  Each core builds
its partial GroupSum table, blends the full batch against it, and the host
sums the 8 partial [B,2] outputs.
"""

from contextlib import ExitStack

import ml_dtypes
import numpy as np

import concourse.bacc as bacc
import concourse.mybir as mybir
import concourse.tile as tile
from concourse.bass_utils import run_bass_kernel_spmd

F32 = mybir.dt.float32
BF16 = mybir.dt.bfloat16
FP8 = mybir.dt.float8e4
ADD = mybir.AluOpType.add
SUB = mybir.AluOpType.subtract
MUL = mybir.AluOpType.mult
GT = mybir.AluOpType.is_gt
X = mybir.AxisListType.X
EXP = mybir.ActivationFunctionType.Exp

N_CORES = 8
B, K, L = 8192, 4096, 6
NS = [32, 16, 8, 4, 2, 1]            # streams per layer
FO = np.cumsum([0] + NS).tolist()    # stream offsets by layer
COLB = [f * 4 for f in FO]           # column base per layer
NCH = 32                             # 8-column chunks (incl. half-chunk of pad)
BROW = B // 128

_compiled = None


def _build_program(u_on_gp=True):
    nc = bacc.Bacc("TRN2", target_bir_lowering=False, debug=False,
                   num_devices=N_CORES)
    wallin = nc.dram_tensor("wallin", [128, 4096], FP8, kind="ExternalInput").ap()
    abkin = nc.dram_tensor("abkin", [128, 1194], BF16, kind="ExternalInput").ap()
    out = nc.dram_tensor("out", [B, 2], F32, kind="ExternalOutput").ap()

    with tile.TileContext(nc) as tc:
        with ExitStack() as ctx:
            p = ctx.enter_context(tc.tile_pool(name="p", bufs=1))
            psp = ctx.enter_context(tc.tile_pool(name="ps", bufs=1, space="PSUM"))

            # ---- input DMAs (wall first: it gates the whole pipeline) ----
            wall = p.tile([128, 4096], FP8)
            nc.sync.dma_start(wall[:, 0:1024], wallin[:, 0:1024])
            nc.sync.dma_start(wall[:, 1024:4096], wallin[:, 1024:4096])
            abk = p.tile([128, 1194], BF16)
            nc.gpsimd.dma_start(abk[:], abkin[:])

            av = abk[:, 0:512].rearrange("p (q c) -> p q c", c=128)
            bv = abk[:, 512:1024].rearrange("p (q c) -> p q c", c=128)
            xv = abk[:, 1024:1152].rearrange("p (a c) -> p a c", c=2)
            kmv = abk[:, 1154:1194]

            # ones for the table-broadcast matmul, built on device
            onesb = p.tile([128, 128], BF16)
            nc.gpsimd.memset(onesb[:], 1.0)

            # blend prep on DVE while weights are in flight
            s0 = p.tile([128, BROW], F32)
            nc.vector.tensor_scalar(s0[:], xv[:, :, 0], 0.0, None, op0=GT)
            s1 = p.tile([128, BROW], F32)
            nc.vector.tensor_scalar(s1[:], xv[:, :, 1], 0.0, None, op0=GT)
            t01 = p.tile([128, BROW], F32)
            nc.vector.tensor_tensor(t01[:], s0[:], s1[:], op=MUL)

            # ---- exp on ACT, 4 chunks ----
            E = p.tile([128, 4096], BF16)
            for k in range(4):
                nc.scalar.activation(E[:, k * 1024:(k + 1) * 1024],
                                     wall[:, k * 1024:(k + 1) * 1024], EXP)

            # ---- coefficient matmuls: one per 8-column chunk ----
            psb = [psp.tile([128, 320], F32, tag=f"pb{b}", name=f"pb{b}")
                   for b in range(4)]
            for c in range(NCH):
                b, s = c // 8, c % 8
                nc.tensor.matmul(psb[b][:, s * 40:(s + 1) * 40],
                                 E[:, c * 128:(c + 1) * 128], kmv,
                                 start=True, stop=True)

            # ---- per-bank: r = 1/D, then normalize+convert coefficients ----
            # slabK[p, k, col]: dense NORMALIZED coefficient planes (c_k / D)
            slabK = p.tile([128, 4, 256], BF16)
            rall = p.tile([128, 256], F32)

            def coeff_bank(b):
                pv = psb[b][:].rearrange("p (s k g) -> p k s g", k=5, g=8)
                rv = rall[:, b * 64:(b + 1) * 64].rearrange("p (s g) -> p s g", g=8)
                nc.vector.reciprocal(rv, pv[:, 4])
                outv = slabK[:, :, b * 64:(b + 1) * 64].rearrange(
                    "p k (s g) -> p k s g", g=8)
                rb = rall[:, b * 64:(b + 1) * 64].rearrange(
                    "p (s g) -> p s g", g=8).unsqueeze(1).broadcast_to(
                    [128, 4, 8, 8])
                nc.vector.tensor_tensor(outv, pv[:, 0:4], rb, op=MUL)

            H = {}
            for l in range(L):
                n = COLB[l + 1] - COLB[l]
                H[l] = p.tile([128, 4, n], BF16, tag=f"H{l}", name=f"H{l}")

            def bc(apl, n):
                return apl.unsqueeze(1).broadcast_to([128, 4, n])

            u_eng = nc.gpsimd if u_on_gp else nc.vector

            def eval_piece(l, lo, hi, tag):
                n = hi - lo
                if l == 0:
                    A = av[:, :, lo:hi]
                    Bv = bv[:, :, lo:hi]
                else:
                    Hp = H[l - 1]
                    A = Hp[:, :, 0:n]
                    Bv = Hp[:, :, n:2 * n]
                c0b = bc(slabK[:, 0, lo:hi], n)
                c1b = bc(slabK[:, 1, lo:hi], n)
                c2b = bc(slabK[:, 2, lo:hi], n)
                c3b = bc(slabK[:, 3, lo:hi], n)
                llo = lo - COLB[l]
                Hv = H[l][:, :, llo:llo + n]
                t = p.tile([128, 4, n], BF16, tag=f"t{tag}", name=f"t{tag}")
                u = p.tile([128, 4, n], BF16, tag=f"u{tag}", name=f"u{tag}")
                nc.vector.tensor_tensor(t[:], Bv, c3b, op=MUL)
                u_eng.tensor_tensor(u[:], Bv, c2b, op=MUL)
                nc.vector.tensor_tensor(t[:], t[:], c1b, op=ADD)
                u_eng.tensor_tensor(u[:], u[:], c0b, op=ADD)
                m = p.tile([128, 4, n], BF16, tag=f"m{tag}", name=f"m{tag}")
                nc.vector.tensor_tensor(m[:], t[:], A, op=MUL)
                nc.vector.tensor_tensor(Hv, m[:], u[:], op=ADD)

            # ---- interleaved schedule ----
            coeff_bank(0)
            eval_piece(0, 0, 64, "l0a")
            coeff_bank(1)
            eval_piece(0, 64, 128, "l0b")
            coeff_bank(2)
            eval_piece(1, 128, 192, "l1")
            coeff_bank(3)
            eval_piece(2, 192, 224, "l2")
            eval_piece(3, 224, 240, "l3")
            eval_piece(4, 240, 248, "l4")
            eval_piece(5, 248, 252, "l5")

            # ---- partial GroupSum table (H5 is already normalized) ----
            Hred = p.tile([128, 4], F32)
            nc.vector.tensor_reduce(Hred[:], H[5][:], axis=X, op=ADD)

            gpt = p.tile([128, 4], BF16)
            up = p.tile([128, 1], F32)
            nc.vector.tensor_copy(gpt[:, 0:1], Hred[:, 0:1])
            nc.vector.tensor_tensor(gpt[:, 1:2], Hred[:, 1:2], Hred[:, 0:1], op=SUB)
            nc.vector.tensor_tensor(gpt[:, 2:3], Hred[:, 2:3], Hred[:, 0:1], op=SUB)
            nc.vector.tensor_tensor(up[:], Hred[:, 3:4], Hred[:, 1:2], op=SUB)
            nc.vector.tensor_tensor(gpt[:, 3:4], up[:], gpt[:, 2:3], op=SUB)

            psg = psp.tile([128, 4], F32)
            nc.tensor.matmul(psg[:], onesb[:], gpt[:], start=True, stop=True)
            g = p.tile([128, 4], F32)
            nc.scalar.copy(g[:], psg[:])

            # ---- per-row blend of the full batch ----
            ev = p.tile([128, BROW], F32)
            nc.vector.tensor_scalar(ev[:], s0[:], g[:, 1:2], g[:, 0:1],
                                    op0=MUL, op1=ADD)
            z1 = p.tile([128, BROW], F32)
            nc.vector.scalar_tensor_tensor(z1[:], s1[:], g[:, 2:3], ev[:],
                                           op0=MUL, op1=ADD)
            z2 = p.tile([128, BROW], F32)
            nc.vector.scalar_tensor_tensor(z2[:], t01[:], g[:, 3:4], z1[:],
                                           op0=MUL, op1=ADD)

            osb = p.tile([128, BROW, 2], F32)
            nc.vector.tensor_tensor(osb[:, :, 0], z2[:],
                                    abk[:, 1152:1153].broadcast_to([128, BROW]),
                                    op=MUL)
            nc.vector.tensor_tensor(osb[:, :, 1], z2[:],
                                    abk[:, 1153:1154].broadcast_to([128, BROW]),
                                    op=MUL)
            nc.sync.dma_start(out.rearrange("(p a) c -> p a c", p=128), osb[:])

    nc.compile()
    return nc


def _host_blobs(x, w0, ws, idx0, idxs):
    """Compose the stream tree and build per-core input blobs."""
    x = np.asarray(x, np.float32)
    Wl = [np.asarray(w0, np.float32)] + [np.asarray(ws[i], np.float32)
                                         for i in range(L - 1)]
    Il = [np.asarray(idx0, np.int64)] + [np.asarray(idxs[i], np.int64)
                                         for i in range(L - 1)]

    S = [None] * L
    S[L - 1] = [np.arange(K)]
    for l in range(L - 1, 0, -1):
        S[l - 1] = [Il[l][0][P] for P in S[l]] + [Il[l][1][P] for P in S[l]]

    # weights in column order: wtmp[core, col, p, i], col = 4*stream + j
    wtmp = np.zeros((N_CORES, 256, 128, 16), np.float32)
    for l in range(L):
        for s in range(NS[l]):
            sg = FO[l] + s
            pw = Wl[l][S[l][s]].reshape(N_CORES, 4, 128, 16)
            wtmp[:, sg * 4:(sg + 1) * 4] = pw
    # wall[core, g*16+i, c*128+p], col = c*8+g
    wt = wtmp.reshape(N_CORES, 32, 8, 128, 16)
    wall = np.ascontiguousarray(
        wt.transpose(0, 2, 4, 1, 3).reshape(N_CORES, 128, 4096))

    # layer-0 pattern inputs, pattern-major: a0[core, p, q*128 + col]
    q = np.arange(4)
    msel0 = np.zeros((N_CORES, 128, 128), np.int64)  # [core, col, p]
    msel1 = np.zeros((N_CORES, 128, 128), np.int64)
    for s in range(NS[0]):
        idx = S[0][s].reshape(N_CORES, 4, 128)
        msel0[:, s * 4:(s + 1) * 4] = Il[0][0][idx]
        msel1[:, s * 4:(s + 1) * 4] = Il[0][1][idx]
    a0 = (q[None, :, None, None] >> msel0[:, None, :, :]) & 1   # [core,q,col,p]
    b0 = (q[None, :, None, None] >> msel1[:, None, :, :]) & 1
    a0 = a0.transpose(0, 3, 1, 2).reshape(N_CORES, 128, 512)    # [core,p,(q,col)]
    b0 = b0.transpose(0, 3, 1, 2).reshape(N_CORES, 128, 512)

    # constant 16->5 coefficient matrix, block-diagonal over 8 groups
    i16 = np.arange(16)
    t11, t10 = i16 & 1, (i16 >> 1) & 1
    t01, t00 = (i16 >> 2) & 1, (i16 >> 3) & 1
    KC = np.stack([t00, t10 - t00, t01 - t00,
                   t11 - t10 - t01 + t00, np.ones(16, np.int64)], 1)  # [16,5]
    kb = np.zeros((8, 16, 5, 8), np.float32)
    for gidx in range(8):
        kb[gidx, :, :, gidx] = KC
    kblob = kb.reshape(128, 40)

    xpart = np.ascontiguousarray(x.reshape(128, 128))

    in_maps = []
    for ci in range(N_CORES):
        cls = np.array([1.0, 0.0] if ci < N_CORES // 2 else [0.0, 1.0],
                       np.float32)
        abkm = np.concatenate(
            [a0[ci], b0[ci], xpart, np.tile(cls, (128, 1)), kblob], axis=1)
        in_maps.append({
            "wallin": wall[ci].astype(ml_dtypes.float8_e4m3fn),
            "abkin": np.ascontiguousarray(abkm).astype(ml_dtypes.bfloat16),
        })
    return in_maps


def run(inputs, trace=False, trace_kwargs=None):
    global _compiled
    if _compiled is None:
        _compiled = _build_program()
    nc = _compiled
    in_maps = _host_blobs(inputs["x"], inputs["w0"], inputs["ws"],
                          inputs["idx0"], inputs["idxs"])
    res = run_bass_kernel_spmd(nc, in_maps, core_ids=list(range(N_CORES)),
                               trace=trace, **(trace_kwargs or {}))
    total = np.zeros((B, 2), np.float32)
    for ci in range(N_CORES):
        total += res.results[ci]["out"]
    return total, res


def kernel(x, w0, ws, idx0, idxs):
    out, _ = run({"x": x, "w0": w0, "ws": ws, "idx0": idx0, "idxs": idxs})
    return out


# revision 12
# speedup vs baseline: 1.4297x; 1.0740x over previous
"""Trainium2 Bass kernel for nn_DiffLogicPBF (difflogic network).

Algorithm
---------
The network input is binarized to 2 bits, so every batch row's activation
trajectory takes one of only 4 "patterns".  We evaluate the network on the 4
patterns instead of 8192 rows, then blend per-row.

The per-layer gathers are composed on the host into a stream tree (layer l is
evaluated 2^(5-l) times in permuted orders, 63 streams total), so the device
does gather-free elementwise work only.  Weights arrive pre-permuted, in fp8.

Device pipeline per core (512 neurons x 63 streams = 252 columns of 128):
  exp(w) on ACT  ->  PE matmul with a constant 16->5 matrix that computes the
  multilinear coefficients c0..c3 and the softmax denominator D per neuron
  (one matmul per 1024-neuron chunk, E as the stationary operand so the
  output lands neuron-major)  ->  r = 1/D per column, then one fused
  PSUM->SBUF multiply that both converts and NORMALIZES the coefficients
  (slab = psum * r), so the eval needs no divisions at all  ->  elementwise
  multilinear evaluation over the 4 patterns, 6 tensor_tensor ops per layer.

Sharding: neurons are split across the 8 cores (512 each).  Each core builds
its partial GroupSum table, blends the full batch against it, and the host
sums the 8 partial [B,2] outputs.
"""

from contextlib import ExitStack

import ml_dtypes
import numpy as np

import concourse.bacc as bacc
import concourse.mybir as mybir
import concourse.tile as tile
from concourse.bass_utils import run_bass_kernel_spmd

F32 = mybir.dt.float32
BF16 = mybir.dt.bfloat16
FP8 = mybir.dt.float8e4
ADD = mybir.AluOpType.add
SUB = mybir.AluOpType.subtract
MUL = mybir.AluOpType.mult
GT = mybir.AluOpType.is_gt
X = mybir.AxisListType.X
EXP = mybir.ActivationFunctionType.Exp

N_CORES = 8
B, K, L = 8192, 4096, 6
NS = [32, 16, 8, 4, 2, 1]            # streams per layer
FO = np.cumsum([0] + NS).tolist()    # stream offsets by layer
COLB = [f * 4 for f in FO]           # column base per layer
NCH = 32                             # 8-column chunks (incl. half-chunk of pad)
BROW = B // 128

_compiled = None


def _build_program(u_on_gp=True):
    nc = bacc.Bacc("TRN2", target_bir_lowering=False, debug=False,
                   num_devices=N_CORES)
    wallin = nc.dram_tensor("wallin", [128, 4096], FP8, kind="ExternalInput").ap()
    abin = nc.dram_tensor("abin", [128, 1024], BF16, kind="ExternalInput").ap()
    xkin = nc.dram_tensor("xkin", [128, 170], BF16, kind="ExternalInput").ap()
    out = nc.dram_tensor("out", [B, 2], F32, kind="ExternalOutput").ap()

    with tile.TileContext(nc) as tc:
        with ExitStack() as ctx:
            p = ctx.enter_context(tc.tile_pool(name="p", bufs=1))
            psp = ctx.enter_context(tc.tile_pool(name="ps", bufs=1, space="PSUM"))

            # ---- input DMAs (wall first: it gates the whole pipeline) ----
            wall = p.tile([128, 4096], FP8)
            nc.sync.dma_start(wall[:, 0:1024], wallin[:, 0:1024])
            nc.sync.dma_start(wall[:, 1024:4096], wallin[:, 1024:4096])
            ab = p.tile([128, 1024], BF16)
            nc.gpsimd.dma_start(ab[:], abin[:])
            xk = p.tile([128, 170], BF16)
            nc.scalar.dma_start(xk[:], xkin[:])

            av = ab[:, 0:512].rearrange("p (q c) -> p q c", c=128)
            bv = ab[:, 512:1024].rearrange("p (q c) -> p q c", c=128)
            xv = xk[:, 0:128].rearrange("p (a c) -> p a c", c=2)
            kmv = xk[:, 130:170]

            # ones for the table-broadcast matmul, built on device
            onesb = p.tile([128, 128], BF16)
            nc.gpsimd.memset(onesb[:], 1.0)

            # blend prep on DVE while weights are in flight
            s0 = p.tile([128, BROW], F32)
            nc.vector.tensor_scalar(s0[:], xv[:, :, 0], 0.0, None, op0=GT)
            s1 = p.tile([128, BROW], F32)
            nc.vector.tensor_scalar(s1[:], xv[:, :, 1], 0.0, None, op0=GT)
            t01 = p.tile([128, BROW], F32)
            nc.vector.tensor_tensor(t01[:], s0[:], s1[:], op=MUL)

            # ---- exp on ACT, 4 chunks ----
            E = p.tile([128, 4096], BF16)
            for k in range(4):
                nc.scalar.activation(E[:, k * 1024:(k + 1) * 1024],
                                     wall[:, k * 1024:(k + 1) * 1024], EXP)

            # ---- coefficient matmuls: one per 8-column chunk ----
            psb = [psp.tile([128, 320], F32, tag=f"pb{b}", name=f"pb{b}")
                   for b in range(4)]
            for c in range(NCH):
                b, s = c // 8, c % 8
                nc.tensor.matmul(psb[b][:, s * 40:(s + 1) * 40],
                                 E[:, c * 128:(c + 1) * 128], kmv,
                                 start=True, stop=True)

            # ---- per-bank: r = 1/D, then normalize+convert coefficients ----
            # slabK[p, k, col]: dense NORMALIZED coefficient planes (c_k / D)
            slabK = p.tile([128, 4, 256], BF16)
            rall = p.tile([128, 256], F32)

            def coeff_bank(b):
                # psum chunk layout is g-major: f = g*5 + k, so the D plane
                # is a single stride-5 run and r fits a rank-2 custom-DVE AP
                pv = psb[b][:].rearrange("p (s g k) -> p k s g", k=5, g=8)
                dflat = psb[b][:].rearrange("p (sg k) -> p sg k", k=5)
                rv = rall[:, b * 64:(b + 1) * 64]
                nc.vector.reciprocal_approx_fast(rv, dflat[:, :, 4])
                outv = slabK[:, :, b * 64:(b + 1) * 64].rearrange(
                    "p k (s g) -> p k s g", g=8)
                rb = rall[:, b * 64:(b + 1) * 64].rearrange(
                    "p (s g) -> p s g", g=8).unsqueeze(1).broadcast_to(
                    [128, 4, 8, 8])
                nc.vector.tensor_tensor(outv, pv[:, 0:4], rb, op=MUL)

            H = {}
            for l in range(L):
                n = COLB[l + 1] - COLB[l]
                H[l] = p.tile([128, 4, n], BF16, tag=f"H{l}", name=f"H{l}")

            def bc(apl, n):
                return apl.unsqueeze(1).broadcast_to([128, 4, n])

            u_eng = nc.gpsimd if u_on_gp else nc.vector

            def eval_piece(l, lo, hi, tag):
                n = hi - lo
                if l == 0:
                    A = av[:, :, lo:hi]
                    Bv = bv[:, :, lo:hi]
                else:
                    Hp = H[l - 1]
                    A = Hp[:, :, 0:n]
                    Bv = Hp[:, :, n:2 * n]
                c0b = bc(slabK[:, 0, lo:hi], n)
                c1b = bc(slabK[:, 1, lo:hi], n)
                c2b = bc(slabK[:, 2, lo:hi], n)
                c3b = bc(slabK[:, 3, lo:hi], n)
                llo = lo - COLB[l]
                Hv = H[l][:, :, llo:llo + n]
                t = p.tile([128, 4, n], BF16, tag=f"t{tag}", name=f"t{tag}")
                u = p.tile([128, 4, n], BF16, tag=f"u{tag}", name=f"u{tag}")
                nc.vector.tensor_tensor(t[:], Bv, c3b, op=MUL)
                u_eng.tensor_tensor(u[:], Bv, c2b, op=MUL)
                nc.vector.tensor_tensor(t[:], t[:], c1b, op=ADD)
                u_eng.tensor_tensor(u[:], u[:], c0b, op=ADD)
                m = p.tile([128, 4, n], BF16, tag=f"m{tag}", name=f"m{tag}")
                nc.vector.tensor_tensor(m[:], t[:], A, op=MUL)
                nc.vector.tensor_tensor(Hv, m[:], u[:], op=ADD)

            # ---- interleaved schedule ----
            coeff_bank(0)
            eval_piece(0, 0, 64, "l0a")
            coeff_bank(1)
            eval_piece(0, 64, 128, "l0b")
            coeff_bank(2)
            eval_piece(1, 128, 192, "l1")
            coeff_bank(3)
            eval_piece(2, 192, 224, "l2")
            eval_piece(3, 224, 240, "l3")
            eval_piece(4, 240, 248, "l4")
            eval_piece(5, 248, 252, "l5")

            # ---- partial GroupSum table (H5 is already normalized) ----
            Hred = p.tile([128, 4], F32)
            nc.vector.tensor_reduce(Hred[:], H[5][:], axis=X, op=ADD)

            gpt = p.tile([128, 4], BF16)
            up = p.tile([128, 1], F32)
            nc.vector.tensor_copy(gpt[:, 0:1], Hred[:, 0:1])
            nc.vector.tensor_tensor(gpt[:, 1:2], Hred[:, 1:2], Hred[:, 0:1], op=SUB)
            nc.vector.tensor_tensor(gpt[:, 2:3], Hred[:, 2:3], Hred[:, 0:1], op=SUB)
            nc.vector.tensor_tensor(up[:], Hred[:, 3:4], Hred[:, 1:2], op=SUB)
            nc.vector.tensor_tensor(gpt[:, 3:4], up[:], gpt[:, 2:3], op=SUB)

            psg = psp.tile([128, 4], F32)
            nc.tensor.matmul(psg[:], onesb[:], gpt[:], start=True, stop=True)

            # ---- per-row blend of the full batch (table read from PSUM) ----
            ev = p.tile([128, BROW], F32)
            nc.vector.tensor_scalar(ev[:], s0[:], psg[:, 1:2], psg[:, 0:1],
                                    op0=MUL, op1=ADD)
            z1 = p.tile([128, BROW], F32)
            nc.vector.scalar_tensor_tensor(z1[:], s1[:], psg[:, 2:3], ev[:],
                                           op0=MUL, op1=ADD)
            z2 = p.tile([128, BROW], F32)
            nc.vector.scalar_tensor_tensor(z2[:], t01[:], psg[:, 3:4], z1[:],
                                           op0=MUL, op1=ADD)

            osb = p.tile([128, BROW, 2], F32)
            nc.vector.tensor_tensor(osb[:, :, 0], z2[:],
                                    xk[:, 128:129].broadcast_to([128, BROW]),
                                    op=MUL)
            nc.vector.tensor_tensor(osb[:, :, 1], z2[:],
                                    xk[:, 129:130].broadcast_to([128, BROW]),
                                    op=MUL)
            nc.sync.dma_start(out.rearrange("(p a) c -> p a c", p=128), osb[:])

    nc.compile()
    return nc


def _host_blobs(x, w0, ws, idx0, idxs):
    """Compose the stream tree and build per-core input blobs."""
    x = np.asarray(x, np.float32)
    Wl = [np.asarray(w0, np.float32)] + [np.asarray(ws[i], np.float32)
                                         for i in range(L - 1)]
    Il = [np.asarray(idx0, np.int64)] + [np.asarray(idxs[i], np.int64)
                                         for i in range(L - 1)]

    S = [None] * L
    S[L - 1] = [np.arange(K)]
    for l in range(L - 1, 0, -1):
        S[l - 1] = [Il[l][0][P] for P in S[l]] + [Il[l][1][P] for P in S[l]]

    # weights in column order: wtmp[core, col, p, i], col = 4*stream + j
    wtmp = np.zeros((N_CORES, 256, 128, 16), np.float32)
    for l in range(L):
        for s in range(NS[l]):
            sg = FO[l] + s
            pw = Wl[l][S[l][s]].reshape(N_CORES, 4, 128, 16)
            wtmp[:, sg * 4:(sg + 1) * 4] = pw
    # wall[core, g*16+i, c*128+p], col = c*8+g
    wt = wtmp.reshape(N_CORES, 32, 8, 128, 16)
    wall = np.ascontiguousarray(
        wt.transpose(0, 2, 4, 1, 3).reshape(N_CORES, 128, 4096))

    # layer-0 pattern inputs, pattern-major: a0[core, p, q*128 + col]
    q = np.arange(4)
    msel0 = np.zeros((N_CORES, 128, 128), np.int64)  # [core, col, p]
    msel1 = np.zeros((N_CORES, 128, 128), np.int64)
    for s in range(NS[0]):
        idx = S[0][s].reshape(N_CORES, 4, 128)
        msel0[:, s * 4:(s + 1) * 4] = Il[0][0][idx]
        msel1[:, s * 4:(s + 1) * 4] = Il[0][1][idx]
    a0 = (q[None, :, None, None] >> msel0[:, None, :, :]) & 1   # [core,q,col,p]
    b0 = (q[None, :, None, None] >> msel1[:, None, :, :]) & 1
    a0 = a0.transpose(0, 3, 1, 2).reshape(N_CORES, 128, 512)    # [core,p,(q,col)]
    b0 = b0.transpose(0, 3, 1, 2).reshape(N_CORES, 128, 512)

    # constant 16->5 coefficient matrix, block-diagonal over 8 groups
    i16 = np.arange(16)
    t11, t10 = i16 & 1, (i16 >> 1) & 1
    t01, t00 = (i16 >> 2) & 1, (i16 >> 3) & 1
    KC = np.stack([t00, t10 - t00, t01 - t00,
                   t11 - t10 - t01 + t00, np.ones(16, np.int64)], 1)  # [16,5]
    kb = np.zeros((8, 16, 8, 5), np.float32)
    for gidx in range(8):
        kb[gidx, :, gidx, :] = KC
    kblob = kb.reshape(128, 40)

    xpart = np.ascontiguousarray(x.reshape(128, 128))

    in_maps = []
    for ci in range(N_CORES):
        cls = np.array([1.0, 0.0] if ci < N_CORES // 2 else [0.0, 1.0],
                       np.float32)
        abm = np.concatenate([a0[ci], b0[ci]], axis=1)
        xkm = np.concatenate([xpart, np.tile(cls, (128, 1)), kblob], axis=1)
        in_maps.append({
            "wallin": wall[ci].astype(ml_dtypes.float8_e4m3fn),
            "abin": np.ascontiguousarray(abm).astype(ml_dtypes.bfloat16),
            "xkin": np.ascontiguousarray(xkm).astype(ml_dtypes.bfloat16),
        })
    return in_maps


def run(inputs, trace=False, trace_kwargs=None):
    global _compiled
    if _compiled is None:
        _compiled = _build_program()
    nc = _compiled
    in_maps = _host_blobs(inputs["x"], inputs["w0"], inputs["ws"],
                          inputs["idx0"], inputs["idxs"])
    res = run_bass_kernel_spmd(nc, in_maps, core_ids=list(range(N_CORES)),
                               trace=trace, **(trace_kwargs or {}))
    total = np.zeros((B, 2), np.float32)
    for ci in range(N_CORES):
        total += res.results[ci]["out"]
    return total, res


def kernel(x, w0, ws, idx0, idxs):
    out, _ = run({"x": x, "w0": w0, "ws": ws, "idx0": idx0, "idxs": idxs})
    return out
